# revision 15
# baseline (speedup 1.0000x reference)
"""APPNP (GCN-normalized propagation) distributed Bass kernel for 8 TRN2 cores.

Strategy (dst-sharded message passing):
  - Nodes sharded across 8 cores (6250/core, padded to 6272 = 49*128 rows).
  - Prologue per core: atom-embedding gather (dma_gather from the flattened
    [9*119, 128] table) summed over 9 feature columns -> h0acc; 3-layer MLP
    on the TensorEngine (f32); produces h (bf16, tile layout) + h0s = 0.1*h
    (f32) per shard.
  - 10 propagation iterations; per iteration:
      AllGather of the 8 bf16 h shards -> full [50176, 128] bf16 h in DRAM,
      dma_gather h[src] for every in-edge of the core's dst shard (edges
      sorted by 64-wide dst block, split lo/hi on src row 32768 for int16
      indices), TensorEngine one-hot(coef) segment-sum matmuls into PSUM,
      blend h_new = PSUM + 0.9*dinv^2 (.) h_prev + 0.1*h0 on the vector
      engine. Self-loop handled via the dinv^2 term (not an edge).
  - Edge coefficients (0.9 * dinv[src]*dinv[dst]) are folded into the
    one-hot S matrices (bf16), so no per-edge vector work is needed.

kernel(**inputs) takes FULL inputs, shards on host, runs the NEFF on cores
0-7, and returns the full [50000, 128] f32 output.
"""

import numpy as np
import ml_dtypes

import concourse.bacc as bacc
import concourse.bass as bass
import concourse.mybir as mybir
import concourse.tile as tile
from concourse.bass_utils import run_bass_kernel_spmd

# Problem constants (hardcoded; must match reference.py)
N_NODES = 50000
N_EDGES = 800000
D = 128
NUM_ITER = 10
NUM_LAYER = 3
ALPHA = 0.1
NUM_ATOM_FEATS = 9
ATOM_VOCAB = 119

NC = 8
SHARD = N_NODES // NC            # 6250
SHARD_PAD = 6272                 # 49 * 128
NCOL = SHARD_PAD // 128          # 49
W = 64                           # dst block width
NBLK = SHARD_PAD // W            # 98
CHUNK = 1024                     # max idxs per dma_gather
HALF_A_ROWS = 3200               # shard rows in exchange buffer A (25 cols)
HALF_B_ROWS = 3072               # shard rows in exchange buffer B (24 cols)
N_A = NC * HALF_A_ROWS           # 25600 (< 32768 -> int16 safe)
N_B = NC * HALF_B_ROWS           # 24576 (< 32768 -> int16 safe)
COLS_A = HALF_A_ROWS // 128      # 25
NQ = 4                           # SWDGE queues

BF16 = mybir.dt.bfloat16
F32 = mybir.dt.float32
I16 = mybir.dt.int16
AF = mybir.ActivationFunctionType


def _wrap_idxs(idx):
    """slot i -> partition i%16 (replicated x8), col i//16."""
    n = idx.shape[0]
    assert n % 16 == 0
    w = idx.reshape(n // 16, 16).T.astype(np.int16)
    return np.ascontiguousarray(np.tile(w, (8, 1)))


def _pad128(a, fill=0):
    n = a.shape[0]
    m = (-n) % 128
    if m == 0:
        return a
    return np.concatenate([a, np.full((m,) + a.shape[1:], fill, a.dtype)])


def _preprocess(edge_index):
    """Host-side graph preprocessing -> per-core structures."""
    src = np.asarray(edge_index[0], dtype=np.int64)
    dst = np.asarray(edge_index[1], dtype=np.int64)
    deg = np.bincount(dst, minlength=N_NODES).astype(np.float64) + 1.0
    dinv = 1.0 / np.sqrt(deg)
    coef = ((1.0 - ALPHA) * dinv[src] * dinv[dst]).astype(np.float32)
    dinv2 = ((1.0 - ALPHA) * dinv * dinv).astype(np.float32)  # self-loop term
    rank = src // SHARD
    r = src % SHARD
    isa = r < HALF_A_ROWS
    srow = np.where(isa, rank * HALF_A_ROWS + r,
                    rank * HALF_B_ROWS + (r - HALF_A_ROWS))

    cores = []
    for c in range(NC):
        m = (dst >= c * SHARD) & (dst < (c + 1) * SHARD)
        nodes = np.arange(SHARD)
        self_isa = nodes < HALF_A_ROWS
        self_row = np.where(self_isa, c * HALF_A_ROWS + nodes,
                            c * HALF_B_ROWS + (nodes - HALF_A_ROWS))
        ldst = np.concatenate([(dst[m] - c * SHARD).astype(np.int64), nodes])
        lsrow = np.concatenate([srow[m], self_row])
        lcoef = np.concatenate([coef[m], dinv2[c * SHARD + nodes]])
        blk = ldst // W
        off = ldst % W
        islo = np.concatenate([isa[m], self_isa])

        streams = {"lo": [], "hi": []}     # list of idx arrays
        s_tiles = []                       # list of [128, W] f32 tile mats
        tiles_by_block = [[] for _ in range(NBLK)]  # (dom, tile_pos_in_stream)
        stream_ntiles = {"lo": 0, "hi": 0}
        for b in range(NBLK):
            bm = blk == b
            for dom, dm in (("lo", islo), ("hi", ~islo)):
                sel = bm & dm
                n = int(sel.sum())
                if n == 0:
                    continue
                idx = _pad128(lsrow[sel].astype(np.int64))
                cf = _pad128(lcoef[sel])
                of = _pad128(off[sel].astype(np.int64))
                ntile = idx.shape[0] // 128
                for t in range(ntile):
                    s = np.zeros((128, W), np.float32)
                    s[np.arange(128), of[t * 128:(t + 1) * 128]] = \
                        cf[t * 128:(t + 1) * 128]
                    tiles_by_block[b].append((dom, stream_ntiles[dom] + t,
                                              len(s_tiles)))
                    s_tiles.append(s)
                streams[dom].append(idx)
                stream_ntiles[dom] += ntile

        lo_idx = (np.concatenate(streams["lo"]) if streams["lo"]
                  else np.zeros(0, np.int64))
        hi_idx = (np.concatenate(streams["hi"]) if streams["hi"]
                  else np.zeros(0, np.int64))
        s_all = (np.stack(s_tiles) if s_tiles
                 else np.zeros((0, 128, W), np.float32))
        # S as SBUF layout [128, ntiles*W]
        s_sb = np.ascontiguousarray(
            s_all.transpose(1, 0, 2).reshape(128, -1)).astype(ml_dtypes.bfloat16)
        cores.append(dict(
            lo_idx=lo_idx, hi_idx=hi_idx, s_sb=s_sb,
            tiles_by_block=tiles_by_block,
            n_lo=lo_idx.shape[0], n_hi=hi_idx.shape[0],
            ntiles=len(s_tiles),
        ))
    return cores


def _chunks(total):
    """Split a stream of `total` slots (multiple of 128) into <=1024 chunks."""
    out = []
    o = 0
    while o < total:
        n = min(CHUNK, total - o)
        out.append((o, n))
        o += n
    return out


def _equalize(cores_meta):
    """Pad per-block/domain tile counts to the max across cores so all cores
    share one instruction stream. Padding tiles gather idx 0 with S=0."""
    # per core: per block, per dom tile count
    cnt = np.zeros((NC, NBLK, 2), np.int64)
    for c, m in enumerate(cores_meta):
        for b in range(NBLK):
            for dom, tpos, sidx in m["tiles_by_block"][b]:
                cnt[c, b, 0 if dom == "lo" else 1] += 1
    mx = cnt.max(axis=0)  # [NBLK, 2]

    new = []
    for c, m in enumerate(cores_meta):
        lo_parts, hi_parts, s_parts = [], [], []
        tiles_by_block = [[] for _ in range(NBLK)]
        lo_idx, hi_idx = m["lo_idx"], m["hi_idx"]
        # existing tiles grouped by block/dom in stream order
        ptr = {"lo": 0, "hi": 0}
        sidx_of = {}
        for b in range(NBLK):
            for dom, tpos, sidx in m["tiles_by_block"][b]:
                sidx_of[(b, dom, tpos)] = sidx
        s_all = m["s_sb"].reshape(128, -1, W)
        lo_nt, hi_nt = 0, 0
        s_n = 0
        for b in range(NBLK):
            for di, dom in enumerate(("lo", "hi")):
                have = [t for t in m["tiles_by_block"][b] if t[0] == dom]
                need = int(mx[b, di])
                for k in range(need):
                    if k < len(have):
                        _, tpos, sidx = have[k]
                        idx_arr = (lo_idx if dom == "lo" else hi_idx)[
                            tpos * 128:(tpos + 1) * 128]
                        s_mat = s_all[:, sidx, :]
                    else:
                        idx_arr = np.zeros(128, np.int64)
                        s_mat = np.zeros((128, W), ml_dtypes.bfloat16)
                    (lo_parts if dom == "lo" else hi_parts).append(idx_arr)
                    s_parts.append(np.asarray(s_mat))
                    nt = lo_nt if dom == "lo" else hi_nt
                    tiles_by_block[b].append((dom, nt, s_n))
                    s_n += 1
                    if dom == "lo":
                        lo_nt += 1
                    else:
                        hi_nt += 1
        lo_cat = (np.concatenate(lo_parts) if lo_parts
                  else np.zeros(0, np.int64))
        hi_cat = (np.concatenate(hi_parts) if hi_parts
                  else np.zeros(0, np.int64))
        s_cat = (np.stack(s_parts) if s_parts
                 else np.zeros((0, 128, W), ml_dtypes.bfloat16))
        s_sb = np.ascontiguousarray(
            np.asarray(s_cat).transpose(1, 0, 2).reshape(128, -1))
        new.append(dict(
            lo_idx=lo_cat, hi_idx=hi_cat, s_sb=s_sb,
            tiles_by_block=tiles_by_block,
            n_lo=lo_cat.shape[0], n_hi=hi_cat.shape[0], ntiles=s_n,
        ))
    return new


def _build_uniform(meta0):
    """Build the (identical-across-cores) program from equalized metadata."""
    n_lo, n_hi, ntiles = meta0["n_lo"], meta0["n_hi"], meta0["ntiles"]
    tiles_by_block = meta0["tiles_by_block"]
    n_emb = NUM_ATOM_FEATS * SHARD_PAD

    nc = bacc.Bacc("TRN2", target_bir_lowering=False, debug=False,
                   num_devices=NC, num_swdge_queues=NQ)

    emb = nc.dram_tensor("emb", [NUM_ATOM_FEATS * ATOM_VOCAB, D], F32,
                         kind="ExternalInput")
    emb_idx = nc.dram_tensor("emb_idx", [128, n_emb // 16], I16,
                             kind="ExternalInput")
    ws = nc.dram_tensor("ws", [NUM_LAYER * D, D], F32, kind="ExternalInput")
    bs = nc.dram_tensor("bs", [NUM_LAYER, D], F32, kind="ExternalInput")
    ident = nc.dram_tensor("ident", [128, 128], F32, kind="ExternalInput")
    idx_lo_d = nc.dram_tensor("idx_lo", [128, max(n_lo, 16) // 16], I16,
                              kind="ExternalInput")
    idx_hi_d = nc.dram_tensor("idx_hi", [128, max(n_hi, 16) // 16], I16,
                              kind="ExternalInput")
    s_d = nc.dram_tensor("s", [128, max(ntiles, 1) * W], BF16,
                         kind="ExternalInput")
    identb_d = nc.dram_tensor("identb", [128, 128], BF16,
                              kind="ExternalInput")
    out_d = nc.dram_tensor("out", [SHARD_PAD, D], F32, kind="ExternalOutput")

    ag_in_a = [nc.dram_tensor(f"ag_in_a{i}", [HALF_A_ROWS, D], BF16,
                              kind="Internal") for i in range(2)]
    ag_in_b = [nc.dram_tensor(f"ag_in_b{i}", [HALF_B_ROWS, D], BF16,
                              kind="Internal") for i in range(2)]
    ag_out_a = [nc.dram_tensor(f"ag_out_a{i}", [N_A, D], BF16,
                               kind="Internal", addr_space="Shared")
                for i in range(2)]
    ag_out_b = [nc.dram_tensor(f"ag_out_b{i}", [N_B, D], BF16,
                               kind="Internal", addr_space="Shared")
                for i in range(2)]

    def _emit_ag(which, buf):
        src = (ag_in_a if which == "a" else ag_in_b)[buf]
        dst = (ag_out_a if which == "a" else ag_out_b)[buf]
        nc.gpsimd.collective_compute(
            "AllGather", mybir.AluOpType.bypass,
            replica_groups=[list(range(NC))],
            ins=[src[:].opt()], outs=[dst[:].opt()])

    lo_chunks = _chunks(n_lo)
    hi_chunks = _chunks(n_hi)

    with tile.TileContext(nc) as tc:
      with tc.tile_pool(name="persist", bufs=1) as persist:
        # ---------------- prologue: embedding + MLP ----------------
        with (
            tc.tile_pool(name="pro", bufs=1) as pro,
            tc.tile_pool(name="embg", bufs=4) as embg,
            tc.tile_pool(name="prps", bufs=2, space="PSUM") as prps,
        ):
            h0acc = pro.tile([128, NCOL, D], F32, tag="h0acc")
            eidx = pro.tile([128, n_emb // 16], I16, tag="eidx")
            nc.sync.dma_start(eidx[:], emb_idx[:])
            qn = 0
            for f in range(NUM_ATOM_FEATS):
                for (o, n) in _chunks(SHARD_PAD):
                    g = embg.tile([128, 8, D], F32, tag="eg")
                    so = f * SHARD_PAD + o
                    nc.gpsimd.dma_gather(
                        g[:, 0:n // 128, :], emb[:, :],
                        eidx[:, so // 16:(so + n) // 16], n, n, D,
                        queue_num=qn % NQ)
                    qn += 1
                    dstap = h0acc[:, o // 128:(o + n) // 128, :]
                    if f == 0:
                        nc.vector.tensor_copy(dstap, g[:, 0:n // 128, :])
                    else:
                        nc.vector.tensor_tensor(
                            dstap, dstap, g[:, 0:n // 128, :],
                            op=mybir.AluOpType.add)

            idn = pro.tile([128, 128], F32, tag="idn")
            nc.sync.dma_start(idn[:], ident[:])
            w_sb = pro.tile([128, NUM_LAYER * D], F32, tag="w")
            b_sb = pro.tile([128, NUM_LAYER], F32, tag="b")
            for l in range(NUM_LAYER):
                nc.sync.dma_start(w_sb[:, l * D:(l + 1) * D],
                                  ws[l * D:(l + 1) * D, :])
                nc.sync.dma_start(b_sb[:, l:l + 1],
                                  bs[l:l + 1, :].rearrange("a k -> k a"))

            hT = pro.tile([128, SHARD_PAD], F32, tag="hT")
            hT2 = pro.tile([128, SHARD_PAD], F32, tag="hT2")
            # transpose h0acc tiles into hT
            for cidx in range(NCOL):
                pt = prps.tile([128, 128], F32, tag="pt")
                nc.tensor.transpose(pt[:], h0acc[:, cidx, :], idn[:])
                nc.vector.tensor_copy(hT[:, cidx * 128:(cidx + 1) * 128],
                                      pt[:])
            # 3 MLP layers (f32)
            cur, nxt = hT, hT2
            mlp_chunks = [(o, min(512, SHARD_PAD - o))
                          for o in range(0, SHARD_PAD, 512)]
            for l in range(NUM_LAYER):
                for (o, n) in mlp_chunks:
                    ps = prps.tile([128, 512], F32, tag="mlp")
                    nc.tensor.matmul(ps[:, 0:n],
                                     w_sb[:, l * D:(l + 1) * D],
                                     cur[:, o:o + n], start=True, stop=True)
                    nc.scalar.activation(
                        nxt[:, o:o + n], ps[:, 0:n],
                        AF.Relu if l != NUM_LAYER - 1 else AF.Identity,
                        bias=b_sb[:, l:l + 1])
                cur, nxt = nxt, cur

            # transpose back; produce h (bf16) and h0s = 0.1*h (f32)
            h_sb = persist.tile([128, NCOL, D], BF16, tag="h")
            h0s = persist.tile([128, NCOL, D], BF16, tag="h0s")
            for cidx in range(NCOL):
                pt = prps.tile([128, 128], F32, tag="pt")
                nc.tensor.transpose(pt[:], cur[:, cidx * 128:(cidx + 1) * 128],
                                    idn[:])
                nc.vector.tensor_copy(h_sb[:, cidx, :], pt[:])
                nc.scalar.activation(h0s[:, cidx, :], pt[:], AF.Copy,
                                     scale=ALPHA)
            nc.sync.dma_start(
                ag_in_a[0][:].rearrange("(c p) f -> p c f", p=128),
                h_sb[:, 0:COLS_A, :])
            nc.sync.dma_start(
                ag_in_b[0][:].rearrange("(c p) f -> p c f", p=128),
                h_sb[:, COLS_A:NCOL, :])

        # ---------------- main loop ----------------
        with (
            tc.tile_pool(name="sconst", bufs=1) as sconst,
            tc.tile_pool(name="glo", bufs=8) as glo_pool,
            tc.tile_pool(name="ghi", bufs=8) as ghi_pool,
            tc.tile_pool(name="ps", bufs=8, space="PSUM") as ps_pool,
            tc.tile_pool(name="stage", bufs=4) as stage_pool,
        ):
            s_sb = sconst.tile([128, max(ntiles, 1) * W], BF16, tag="s")
            nc.sync.dma_start(s_sb[:], s_d[:])
            ilo = sconst.tile([128, max(n_lo, 16) // 16], I16, tag="ilo")
            nc.sync.dma_start(ilo[:], idx_lo_d[:])
            ihi = sconst.tile([128, max(n_hi, 16) // 16], I16, tag="ihi")
            nc.sync.dma_start(ihi[:], idx_hi_d[:])
            identb = sconst.tile([128, 128], BF16, tag="identb")
            nc.sync.dma_start(identb[:], identb_d[:])

            _emit_ag("a", 0)
            _emit_ag("b", 0)

            for it in range(NUM_ITER):
                buf = it % 2
                lo_view = ag_out_a[buf][:, :]
                hi_view = ag_out_b[buf][:, :]

                # issue gathers, interleaved a/b
                lo_tiles_bufs = {}
                hi_tiles_bufs = {}
                qn = 0
                li, hi_i = 0, 0
                order = []
                # lead with A-chunks so AG_b completion hides behind them
                while li < 12 and li < len(lo_chunks):
                    order.append(("lo", li)); li += 1
                while li < len(lo_chunks) or hi_i < len(hi_chunks):
                    if li < len(lo_chunks):
                        order.append(("lo", li)); li += 1
                    if hi_i < len(hi_chunks):
                        order.append(("hi", hi_i)); hi_i += 1
                for dom, ci in order:
                    (o, n) = (lo_chunks if dom == "lo" else hi_chunks)[ci]
                    pool = glo_pool if dom == "lo" else ghi_pool
                    view = lo_view if dom == "lo" else hi_view
                    isb = ilo if dom == "lo" else ihi
                    g = pool.tile([128, 8, D], BF16, tag="g" + dom)
                    nc.gpsimd.dma_gather(
                        g[:, 0:n // 128, :], view,
                        isb[:, o // 16:(o + n) // 16], n, n, D,
                        queue_num=qn % NQ)
                    qn += 1
                    (lo_tiles_bufs if dom == "lo" else hi_tiles_bufs)[ci] = g

                # segment-sum matmuls + evict, block-pair by block-pair
                last = it == NUM_ITER - 1
                for p in range(NBLK // 2):
                    col = p
                    psum = ps_pool.tile([128, D], F32, tag="ps")
                    for half in range(2):
                        tl = tiles_by_block[2 * p + half]
                        ph = half * 64
                        # inject h0s (= 0.1*h0) via identity matmul
                        nc.tensor.matmul(
                            psum[ph:ph + 64, :],
                            identb[:, ph:ph + 64],
                            h0s[:, col, :],
                            start=True, stop=(len(tl) == 0))
                        for j, (dom, tpos, sidx) in enumerate(tl):
                            bufs = (lo_tiles_bufs if dom == "lo"
                                    else hi_tiles_bufs)
                            g = bufs[tpos // 8]
                            nc.tensor.matmul(
                                psum[ph:ph + 64, :],
                                s_sb[:, sidx * W:(sidx + 1) * W],
                                g[:, tpos % 8, :],
                                start=False, stop=(j == len(tl) - 1))
                    if last:
                        st = stage_pool.tile([128, D], F32, tag="st")
                        nc.scalar.activation(st[:], psum[:, :], AF.Copy)
                        nc.sync.dma_start(
                            out_d[p * 128:(p + 1) * 128, :], st[:])
                    else:
                        nc.scalar.activation(h_sb[:, col, :], psum[:, :],
                                             AF.Copy)
                    if not last:
                        if p == COLS_A - 1:
                            nc.sync.dma_start(
                                ag_in_a[1 - buf][:].rearrange(
                                    "(c p) f -> p c f", p=128),
                                h_sb[:, 0:COLS_A, :])
                            _emit_ag("a", 1 - buf)
                        elif p == NBLK // 2 - 1:
                            nc.sync.dma_start(
                                ag_in_b[1 - buf][:].rearrange(
                                    "(c p) f -> p c f", p=128),
                                h_sb[:, COLS_A:NCOL, :])
                            _emit_ag("b", 1 - buf)

    nc.compile()
    return nc


_CACHE = {}


def _get_compiled(edge_index):
    key = hash(np.asarray(edge_index).tobytes())
    if key not in _CACHE:
        cores = _preprocess(edge_index)
        cores = _equalize(cores)
        nc = _build_uniform(cores[0])
        _CACHE[key] = (nc, cores)
    return _CACHE[key]


def _make_in_maps(x, atom_emb, Ws, bs, cores_meta):
    x = np.asarray(x)
    emb_t = np.ascontiguousarray(
        np.asarray(atom_emb, dtype=np.float32).reshape(
            NUM_ATOM_FEATS * ATOM_VOCAB, D))
    ws_t = np.ascontiguousarray(
        np.asarray(Ws, dtype=np.float32).reshape(NUM_LAYER * D, D))
    bs_t = np.ascontiguousarray(np.asarray(bs, dtype=np.float32))
    ident = np.eye(128, dtype=np.float32)

    in_maps = []
    for c, m in enumerate(cores_meta):
        # embedding idx: per feature stream of SHARD_PAD slots
        ei = np.zeros(NUM_ATOM_FEATS * SHARD_PAD, np.int64)
        xs = x[c * SHARD:(c + 1) * SHARD]  # [SHARD, 9]
        for f in range(NUM_ATOM_FEATS):
            ei[f * SHARD_PAD:f * SHARD_PAD + SHARD] = \
                f * ATOM_VOCAB + xs[:, f]
        lo = m["lo_idx"] if m["n_lo"] else np.zeros(16, np.int64)
        hi_ = m["hi_idx"] if m["n_hi"] else np.zeros(16, np.int64)
        in_maps.append({
            "emb": emb_t,
            "emb_idx": _wrap_idxs(ei),
            "ws": ws_t,
            "bs": bs_t,
            "ident": ident,
            "idx_lo": _wrap_idxs(lo),
            "idx_hi": _wrap_idxs(hi_),
            "s": np.ascontiguousarray(m["s_sb"]),
            "identb": np.eye(128, dtype=ml_dtypes.bfloat16),
        })
    return in_maps


def kernel(x, edge_index, atom_emb, Ws, bs):
    nc, cores_meta = _get_compiled(edge_index)
    in_maps = _make_in_maps(x, atom_emb, Ws, bs, cores_meta)
    res = run_bass_kernel_spmd(nc, in_maps, core_ids=list(range(NC)))
    out = np.concatenate(
        [res.results[c]["out"][:SHARD] for c in range(NC)], axis=0)
    return np.ascontiguousarray(out.astype(np.float32))


def run_profiled(x, edge_index, atom_emb, Ws, bs):
    """Like kernel() but with NTFF profiling; returns (out, exec_time_ns)."""
    import ntff_hook
    ntff_hook.install()
    nc, cores_meta = _get_compiled(edge_index)
    in_maps = _make_in_maps(x, atom_emb, Ws, bs, cores_meta)
    res = run_bass_kernel_spmd(nc, in_maps, core_ids=list(range(NC)),
                               trace=True)
    out = np.concatenate(
        [res.results[c]["out"][:SHARD] for c in range(NC)], axis=0)
    return np.ascontiguousarray(out.astype(np.float32)), res.exec_time_ns


# revision 17
# speedup vs baseline: 1.4390x; 1.4390x over previous
"""APPNP (GCN-normalized propagation) distributed Bass kernel for 8 TRN2 cores.

Strategy (dst-sharded message passing):
  - Nodes sharded across 8 cores (6250/core, padded to 6272 = 49*128 rows).
  - Prologue per core: atom-embedding gather (dma_gather from the flattened
    [9*119, 128] table) summed over 9 feature columns -> h0acc; 3-layer MLP
    on the TensorEngine (f32); produces h (bf16, tile layout) + h0s = 0.1*h
    (bf16) per shard.
  - Per-iteration exchange: each shard is split into an A half (rows 0:3200)
    and B half (3200:6272); two AllGathers produce full bf16 h copies in two
    Shared DRAM buffers of 25600/24576 rows (< 32768, so dma_gather's int16
    indices reach everything). Buffers are double-buffered across iterations
    and AG_a is issued mid-iteration (as soon as the A-half blends finish),
    so the collectives hide behind gather/matmul work.
  - Per iteration: dma_gather h[src] for every in-edge of the core's dst
    shard (incl. self-loops, which are ordinary edges with coef
    0.9*dinv^2), edges grouped by 64-wide dst block into 128-slot tiles;
    TensorEngine segment-sum via one-hot(0.9*coef) S matrices (bf16,
    SBUF-resident) accumulating in PSUM; h0s injected into PSUM with an
    identity matmul; the scalar engine evicts PSUM -> h (bf16) / out (f32).
    The vector engine does no per-edge work at all.
  - Per-block/domain tile counts are equalized across cores so all 8 cores
    run one SPMD instruction stream (padding tiles gather idx 0 with S=0).

kernel(**inputs) takes FULL inputs, shards on host, runs the NEFF on cores
0-7, and returns the full [50000, 128] f32 output.
"""

import numpy as np
import ml_dtypes

import concourse.bacc as bacc
import concourse.bass as bass
import concourse.mybir as mybir
import concourse.tile as tile
from concourse.bass_utils import run_bass_kernel_spmd

# Problem constants (hardcoded; must match reference.py)
N_NODES = 50000
N_EDGES = 800000
D = 128
NUM_ITER = 10
NUM_LAYER = 3
ALPHA = 0.1
NUM_ATOM_FEATS = 9
ATOM_VOCAB = 119

NC = 8
SHARD = N_NODES // NC            # 6250
SHARD_PAD = 6272                 # 49 * 128
NCOL = SHARD_PAD // 128          # 49
W = 64                           # dst block width
NBLK = SHARD_PAD // W            # 98
CHUNK = 1024                     # max idxs per dma_gather
HALF_A_ROWS = 3200               # shard rows in exchange buffer A (25 cols)
HALF_B_ROWS = 3072               # shard rows in exchange buffer B (24 cols)
N_A = NC * HALF_A_ROWS           # 25600 (< 32768 -> int16 safe)
N_B = NC * HALF_B_ROWS           # 24576 (< 32768 -> int16 safe)
COLS_A = HALF_A_ROWS // 128      # 25
NQ = 4                           # SWDGE queues

BF16 = mybir.dt.bfloat16
F32 = mybir.dt.float32
I16 = mybir.dt.int16
AF = mybir.ActivationFunctionType


def _wrap_idxs(idx):
    """slot i -> partition i%16 (replicated x8), col i//16."""
    n = idx.shape[0]
    assert n % 16 == 0
    w = idx.reshape(n // 16, 16).T.astype(np.int16)
    return np.ascontiguousarray(np.tile(w, (8, 1)))


def _pad128(a, fill=0):
    n = a.shape[0]
    m = (-n) % 128
    if m == 0:
        return a
    return np.concatenate([a, np.full((m,) + a.shape[1:], fill, a.dtype)])


def _preprocess(edge_index):
    """Host-side graph preprocessing -> per-core structures."""
    src = np.asarray(edge_index[0], dtype=np.int64)
    dst = np.asarray(edge_index[1], dtype=np.int64)
    deg = np.bincount(dst, minlength=N_NODES).astype(np.float64) + 1.0
    dinv = 1.0 / np.sqrt(deg)
    coef = ((1.0 - ALPHA) * dinv[src] * dinv[dst]).astype(np.float32)
    dinv2 = ((1.0 - ALPHA) * dinv * dinv).astype(np.float32)  # self-loop term
    rank = src // SHARD
    r = src % SHARD
    isa = r < HALF_A_ROWS
    srow = np.where(isa, rank * HALF_A_ROWS + r,
                    rank * HALF_B_ROWS + (r - HALF_A_ROWS))

    cores = []
    for c in range(NC):
        m = (dst >= c * SHARD) & (dst < (c + 1) * SHARD)
        nodes = np.arange(SHARD)
        self_isa = nodes < HALF_A_ROWS
        self_row = np.where(self_isa, c * HALF_A_ROWS + nodes,
                            c * HALF_B_ROWS + (nodes - HALF_A_ROWS))
        ldst = np.concatenate([(dst[m] - c * SHARD).astype(np.int64), nodes])
        lsrow = np.concatenate([srow[m], self_row])
        lcoef = np.concatenate([coef[m], dinv2[c * SHARD + nodes]])
        blk = ldst // W
        off = ldst % W
        islo = np.concatenate([isa[m], self_isa])

        streams = {"lo": [], "hi": []}     # list of idx arrays
        s_tiles = []                       # list of [128, W] f32 tile mats
        tiles_by_block = [[] for _ in range(NBLK)]  # (dom, tile_pos_in_stream)
        stream_ntiles = {"lo": 0, "hi": 0}
        for b in range(NBLK):
            bm = blk == b
            for dom, dm in (("lo", islo), ("hi", ~islo)):
                sel = bm & dm
                n = int(sel.sum())
                if n == 0:
                    continue
                idx = _pad128(lsrow[sel].astype(np.int64))
                cf = _pad128(lcoef[sel])
                of = _pad128(off[sel].astype(np.int64))
                ntile = idx.shape[0] // 128
                for t in range(ntile):
                    s = np.zeros((128, W), np.float32)
                    s[np.arange(128), of[t * 128:(t + 1) * 128]] = \
                        cf[t * 128:(t + 1) * 128]
                    tiles_by_block[b].append((dom, stream_ntiles[dom] + t,
                                              len(s_tiles)))
                    s_tiles.append(s)
                streams[dom].append(idx)
                stream_ntiles[dom] += ntile

        lo_idx = (np.concatenate(streams["lo"]) if streams["lo"]
                  else np.zeros(0, np.int64))
        hi_idx = (np.concatenate(streams["hi"]) if streams["hi"]
                  else np.zeros(0, np.int64))
        s_all = (np.stack(s_tiles) if s_tiles
                 else np.zeros((0, 128, W), np.float32))
        # S as SBUF layout [128, ntiles*W]
        s_sb = np.ascontiguousarray(
            s_all.transpose(1, 0, 2).reshape(128, -1)).astype(ml_dtypes.bfloat16)
        cores.append(dict(
            lo_idx=lo_idx, hi_idx=hi_idx, s_sb=s_sb,
            tiles_by_block=tiles_by_block,
            n_lo=lo_idx.shape[0], n_hi=hi_idx.shape[0],
            ntiles=len(s_tiles),
        ))
    return cores


def _chunks(total):
    """Split a stream of `total` slots (multiple of 128) into <=1024 chunks."""
    out = []
    o = 0
    while o < total:
        n = min(CHUNK, total - o)
        out.append((o, n))
        o += n
    return out


def _equalize(cores_meta):
    """Pad per-block/domain tile counts to the max across cores so all cores
    share one instruction stream. Padding tiles gather idx 0 with S=0."""
    # per core: per block, per dom tile count
    cnt = np.zeros((NC, NBLK, 2), np.int64)
    for c, m in enumerate(cores_meta):
        for b in range(NBLK):
            for dom, tpos, sidx in m["tiles_by_block"][b]:
                cnt[c, b, 0 if dom == "lo" else 1] += 1
    mx = cnt.max(axis=0)  # [NBLK, 2]

    new = []
    for c, m in enumerate(cores_meta):
        lo_parts, hi_parts, s_parts = [], [], []
        tiles_by_block = [[] for _ in range(NBLK)]
        lo_idx, hi_idx = m["lo_idx"], m["hi_idx"]
        # existing tiles grouped by block/dom in stream order
        ptr = {"lo": 0, "hi": 0}
        sidx_of = {}
        for b in range(NBLK):
            for dom, tpos, sidx in m["tiles_by_block"][b]:
                sidx_of[(b, dom, tpos)] = sidx
        s_all = m["s_sb"].reshape(128, -1, W)
        lo_nt, hi_nt = 0, 0
        s_n = 0
        for b in range(NBLK):
            for di, dom in enumerate(("lo", "hi")):
                have = [t for t in m["tiles_by_block"][b] if t[0] == dom]
                need = int(mx[b, di])
                for k in range(need):
                    if k < len(have):
                        _, tpos, sidx = have[k]
                        idx_arr = (lo_idx if dom == "lo" else hi_idx)[
                            tpos * 128:(tpos + 1) * 128]
                        s_mat = s_all[:, sidx, :]
                    else:
                        idx_arr = np.zeros(128, np.int64)
                        s_mat = np.zeros((128, W), ml_dtypes.bfloat16)
                    (lo_parts if dom == "lo" else hi_parts).append(idx_arr)
                    s_parts.append(np.asarray(s_mat))
                    nt = lo_nt if dom == "lo" else hi_nt
                    tiles_by_block[b].append((dom, nt, s_n))
                    s_n += 1
                    if dom == "lo":
                        lo_nt += 1
                    else:
                        hi_nt += 1
        lo_cat = (np.concatenate(lo_parts) if lo_parts
                  else np.zeros(0, np.int64))
        hi_cat = (np.concatenate(hi_parts) if hi_parts
                  else np.zeros(0, np.int64))
        s_cat = (np.stack(s_parts) if s_parts
                 else np.zeros((0, 128, W), ml_dtypes.bfloat16))
        s_sb = np.ascontiguousarray(
            np.asarray(s_cat).transpose(1, 0, 2).reshape(128, -1))
        new.append(dict(
            lo_idx=lo_cat, hi_idx=hi_cat, s_sb=s_sb,
            tiles_by_block=tiles_by_block,
            n_lo=lo_cat.shape[0], n_hi=hi_cat.shape[0], ntiles=s_n,
        ))
    return new


def _build_uniform(meta0):
    """Build the (identical-across-cores) program from equalized metadata."""
    n_lo, n_hi, ntiles = meta0["n_lo"], meta0["n_hi"], meta0["ntiles"]
    tiles_by_block = meta0["tiles_by_block"]
    n_emb = NUM_ATOM_FEATS * SHARD_PAD

    nc = bacc.Bacc("TRN2", target_bir_lowering=False, debug=False,
                   num_devices=NC, num_swdge_queues=NQ)

    emb = nc.dram_tensor("emb", [NUM_ATOM_FEATS * ATOM_VOCAB, D], F32,
                         kind="ExternalInput")
    emb_idx = nc.dram_tensor("emb_idx", [128, n_emb // 16], I16,
                             kind="ExternalInput")
    ws = nc.dram_tensor("ws", [NUM_LAYER * D, D], F32, kind="ExternalInput")
    bs = nc.dram_tensor("bs", [NUM_LAYER, D], F32, kind="ExternalInput")
    ident = nc.dram_tensor("ident", [128, 128], F32, kind="ExternalInput")
    idx_lo_d = nc.dram_tensor("idx_lo", [128, max(n_lo, 16) // 16], I16,
                              kind="ExternalInput")
    idx_hi_d = nc.dram_tensor("idx_hi", [128, max(n_hi, 16) // 16], I16,
                              kind="ExternalInput")
    s_d = nc.dram_tensor("s", [128, max(ntiles, 1) * W], BF16,
                         kind="ExternalInput")
    identb_d = nc.dram_tensor("identb", [128, 128], BF16,
                              kind="ExternalInput")
    out_d = nc.dram_tensor("out", [SHARD_PAD, D], F32, kind="ExternalOutput")

    ag_in_a = [nc.dram_tensor(f"ag_in_a{i}", [HALF_A_ROWS, D], BF16,
                              kind="Internal") for i in range(2)]
    ag_in_b = [nc.dram_tensor(f"ag_in_b{i}", [HALF_B_ROWS, D], BF16,
                              kind="Internal") for i in range(2)]
    ag_out_a = [nc.dram_tensor(f"ag_out_a{i}", [N_A, D], BF16,
                               kind="Internal", addr_space="Shared")
                for i in range(2)]
    ag_out_b = [nc.dram_tensor(f"ag_out_b{i}", [N_B, D], BF16,
                               kind="Internal", addr_space="Shared")
                for i in range(2)]

    def _emit_ag(which, buf):
        src = (ag_in_a if which == "a" else ag_in_b)[buf]
        dst = (ag_out_a if which == "a" else ag_out_b)[buf]
        nc.gpsimd.collective_compute(
            "AllGather", mybir.AluOpType.bypass,
            replica_groups=[list(range(NC))],
            ins=[src[:].opt()], outs=[dst[:].opt()])

    lo_chunks = _chunks(n_lo)
    hi_chunks = _chunks(n_hi)

    with tile.TileContext(nc) as tc:
      with tc.tile_pool(name="persist", bufs=1) as persist:
        # ---------------- prologue: embedding + MLP ----------------
        with (
            tc.tile_pool(name="pro", bufs=1) as pro,
            tc.tile_pool(name="embg", bufs=4) as embg,
            tc.tile_pool(name="prps", bufs=2, space="PSUM") as prps,
        ):
            h0acc = pro.tile([128, NCOL, D], F32, tag="h0acc")
            eidx = pro.tile([128, n_emb // 16], I16, tag="eidx")
            nc.sync.dma_start(eidx[:], emb_idx[:])
            qn = 0
            for f in range(NUM_ATOM_FEATS):
                for (o, n) in _chunks(SHARD_PAD):
                    g = embg.tile([128, 8, D], F32, tag="eg")
                    so = f * SHARD_PAD + o
                    nc.gpsimd.dma_gather(
                        g[:, 0:n // 128, :], emb[:, :],
                        eidx[:, so // 16:(so + n) // 16], n, n, D,
                        queue_num=qn % NQ)
                    qn += 1
                    dstap = h0acc[:, o // 128:(o + n) // 128, :]
                    if f == 0:
                        nc.vector.tensor_copy(dstap, g[:, 0:n // 128, :])
                    else:
                        nc.vector.tensor_tensor(
                            dstap, dstap, g[:, 0:n // 128, :],
                            op=mybir.AluOpType.add)

            idn = pro.tile([128, 128], F32, tag="idn")
            nc.sync.dma_start(idn[:], ident[:])
            w_sb = pro.tile([128, NUM_LAYER * D], F32, tag="w")
            b_sb = pro.tile([128, NUM_LAYER], F32, tag="b")
            for l in range(NUM_LAYER):
                nc.sync.dma_start(w_sb[:, l * D:(l + 1) * D],
                                  ws[l * D:(l + 1) * D, :])
                nc.sync.dma_start(b_sb[:, l:l + 1],
                                  bs[l:l + 1, :].rearrange("a k -> k a"))

            hT = pro.tile([128, SHARD_PAD], F32, tag="hT")
            hT2 = pro.tile([128, SHARD_PAD], F32, tag="hT2")
            # transpose h0acc tiles into hT
            for cidx in range(NCOL):
                pt = prps.tile([128, 128], F32, tag="pt")
                nc.tensor.transpose(pt[:], h0acc[:, cidx, :], idn[:])
                nc.vector.tensor_copy(hT[:, cidx * 128:(cidx + 1) * 128],
                                      pt[:])
            # 3 MLP layers (f32)
            cur, nxt = hT, hT2
            mlp_chunks = [(o, min(512, SHARD_PAD - o))
                          for o in range(0, SHARD_PAD, 512)]
            for l in range(NUM_LAYER):
                for (o, n) in mlp_chunks:
                    ps = prps.tile([128, 512], F32, tag="mlp")
                    nc.tensor.matmul(ps[:, 0:n],
                                     w_sb[:, l * D:(l + 1) * D],
                                     cur[:, o:o + n], start=True, stop=True)
                    nc.scalar.activation(
                        nxt[:, o:o + n], ps[:, 0:n],
                        AF.Relu if l != NUM_LAYER - 1 else AF.Identity,
                        bias=b_sb[:, l:l + 1])
                cur, nxt = nxt, cur

            # transpose back; produce h (bf16) and h0s = 0.1*h (f32)
            h_sb = persist.tile([128, NCOL, D], BF16, tag="h")
            h0s = persist.tile([128, NCOL, D], BF16, tag="h0s")
            for cidx in range(NCOL):
                pt = prps.tile([128, 128], F32, tag="pt")
                nc.tensor.transpose(pt[:], cur[:, cidx * 128:(cidx + 1) * 128],
                                    idn[:])
                nc.vector.tensor_copy(h_sb[:, cidx, :], pt[:])
                nc.scalar.activation(h0s[:, cidx, :], pt[:], AF.Copy,
                                     scale=ALPHA)
            nc.sync.dma_start(
                ag_in_a[0][:].rearrange("(c p) f -> p c f", p=128),
                h_sb[:, 0:COLS_A, :])
            nc.sync.dma_start(
                ag_in_b[0][:].rearrange("(c p) f -> p c f", p=128),
                h_sb[:, COLS_A:NCOL, :])

        # ---------------- main loop ----------------
        with (
            tc.tile_pool(name="sconst", bufs=1) as sconst,
            tc.tile_pool(name="glo", bufs=8) as glo_pool,
            tc.tile_pool(name="ghi", bufs=8) as ghi_pool,
            tc.tile_pool(name="ps", bufs=8, space="PSUM") as ps_pool,
            tc.tile_pool(name="stage", bufs=4) as stage_pool,
        ):
            s_sb = sconst.tile([128, max(ntiles, 1) * W], BF16, tag="s")
            nc.sync.dma_start(s_sb[:], s_d[:])
            ilo = sconst.tile([128, max(n_lo, 16) // 16], I16, tag="ilo")
            nc.sync.dma_start(ilo[:], idx_lo_d[:])
            ihi = sconst.tile([128, max(n_hi, 16) // 16], I16, tag="ihi")
            nc.sync.dma_start(ihi[:], idx_hi_d[:])
            identb = sconst.tile([128, 128], BF16, tag="identb")
            nc.sync.dma_start(identb[:], identb_d[:])

            _emit_ag("a", 0)
            _emit_ag("b", 0)

            for it in range(NUM_ITER):
                buf = it % 2
                lo_view = ag_out_a[buf][:, :]
                hi_view = ag_out_b[buf][:, :]

                # issue gathers, interleaved a/b
                lo_tiles_bufs = {}
                hi_tiles_bufs = {}
                qn = 0
                li, hi_i = 0, 0
                order = []
                while li < len(lo_chunks) or hi_i < len(hi_chunks):
                    if li < len(lo_chunks):
                        order.append(("lo", li)); li += 1
                    if hi_i < len(hi_chunks):
                        order.append(("hi", hi_i)); hi_i += 1
                for dom, ci in order:
                    (o, n) = (lo_chunks if dom == "lo" else hi_chunks)[ci]
                    pool = glo_pool if dom == "lo" else ghi_pool
                    view = lo_view if dom == "lo" else hi_view
                    isb = ilo if dom == "lo" else ihi
                    g = pool.tile([128, 8, D], BF16, tag="g" + dom)
                    nc.gpsimd.dma_gather(
                        g[:, 0:n // 128, :], view,
                        isb[:, o // 16:(o + n) // 16], n, n, D,
                        queue_num=qn % NQ)
                    qn += 1
                    (lo_tiles_bufs if dom == "lo" else hi_tiles_bufs)[ci] = g

                # segment-sum matmuls + evict, block-pair by block-pair
                last = it == NUM_ITER - 1
                for p in range(NBLK // 2):
                    col = p
                    psum = ps_pool.tile([128, D], F32, tag="ps")
                    for half in range(2):
                        tl = tiles_by_block[2 * p + half]
                        ph = half * 64
                        # inject h0s (= 0.1*h0) via identity matmul
                        nc.tensor.matmul(
                            psum[ph:ph + 64, :],
                            identb[:, ph:ph + 64],
                            h0s[:, col, :],
                            start=True, stop=(len(tl) == 0))
                        for j, (dom, tpos, sidx) in enumerate(tl):
                            bufs = (lo_tiles_bufs if dom == "lo"
                                    else hi_tiles_bufs)
                            g = bufs[tpos // 8]
                            nc.tensor.matmul(
                                psum[ph:ph + 64, :],
                                s_sb[:, sidx * W:(sidx + 1) * W],
                                g[:, tpos % 8, :],
                                start=False, stop=(j == len(tl) - 1))
                    if last:
                        st = stage_pool.tile([128, D], F32, tag="st")
                        nc.scalar.activation(st[:], psum[:, :], AF.Copy)
                        nc.sync.dma_start(
                            out_d[p * 128:(p + 1) * 128, :], st[:])
                    else:
                        nc.scalar.activation(h_sb[:, col, :], psum[:, :],
                                             AF.Copy)
                    if not last:
                        if p == COLS_A - 1:
                            nc.sync.dma_start(
                                ag_in_a[1 - buf][:].rearrange(
                                    "(c p) f -> p c f", p=128),
                                h_sb[:, 0:COLS_A, :])
                            _emit_ag("a", 1 - buf)
                        elif p == NBLK // 2 - 1:
                            nc.sync.dma_start(
                                ag_in_b[1 - buf][:].rearrange(
                                    "(c p) f -> p c f", p=128),
                                h_sb[:, COLS_A:NCOL, :])
                            _emit_ag("b", 1 - buf)

    nc.compile()
    return nc


_CACHE = {}


def _get_compiled(edge_index):
    key = hash(np.asarray(edge_index).tobytes())
    if key not in _CACHE:
        cores = _preprocess(edge_index)
        cores = _equalize(cores)
        nc = _build_uniform(cores[0])
        _CACHE[key] = (nc, cores)
    return _CACHE[key]


def _make_in_maps(x, atom_emb, Ws, bs, cores_meta):
    x = np.asarray(x)
    emb_t = np.ascontiguousarray(
        np.asarray(atom_emb, dtype=np.float32).reshape(
            NUM_ATOM_FEATS * ATOM_VOCAB, D))
    ws_t = np.ascontiguousarray(
        np.asarray(Ws, dtype=np.float32).reshape(NUM_LAYER * D, D))
    bs_t = np.ascontiguousarray(np.asarray(bs, dtype=np.float32))
    ident = np.eye(128, dtype=np.float32)

    in_maps = []
    for c, m in enumerate(cores_meta):
        # embedding idx: per feature stream of SHARD_PAD slots
        ei = np.zeros(NUM_ATOM_FEATS * SHARD_PAD, np.int64)
        xs = x[c * SHARD:(c + 1) * SHARD]  # [SHARD, 9]
        for f in range(NUM_ATOM_FEATS):
            ei[f * SHARD_PAD:f * SHARD_PAD + SHARD] = \
                f * ATOM_VOCAB + xs[:, f]
        lo = m["lo_idx"] if m["n_lo"] else np.zeros(16, np.int64)
        hi_ = m["hi_idx"] if m["n_hi"] else np.zeros(16, np.int64)
        in_maps.append({
            "emb": emb_t,
            "emb_idx": _wrap_idxs(ei),
            "ws": ws_t,
            "bs": bs_t,
            "ident": ident,
            "idx_lo": _wrap_idxs(lo),
            "idx_hi": _wrap_idxs(hi_),
            "s": np.ascontiguousarray(m["s_sb"]),
            "identb": np.eye(128, dtype=ml_dtypes.bfloat16),
        })
    return in_maps


def kernel(x, edge_index, atom_emb, Ws, bs):
    nc, cores_meta = _get_compiled(edge_index)
    in_maps = _make_in_maps(x, atom_emb, Ws, bs, cores_meta)
    res = run_bass_kernel_spmd(nc, in_maps, core_ids=list(range(NC)))
    out = np.concatenate(
        [res.results[c]["out"][:SHARD] for c in range(NC)], axis=0)
    return np.ascontiguousarray(out.astype(np.float32))


def run_profiled(x, edge_index, atom_emb, Ws, bs):
    """Like kernel() but with NTFF profiling; returns (out, exec_time_ns)."""
    import ntff_hook
    ntff_hook.install()
    nc, cores_meta = _get_compiled(edge_index)
    in_maps = _make_in_maps(x, atom_emb, Ws, bs, cores_meta)
    res = run_bass_kernel_spmd(nc, in_maps, core_ids=list(range(NC)),
                               trace=True)
    out = np.concatenate(
        [res.results[c]["out"][:SHARD] for c in range(NC)], axis=0)
    return np.ascontiguousarray(out.astype(np.float32)), res.exec_time_ns


# revision 19
# speedup vs baseline: 1.4634x; 1.0170x over previous
"""APPNP (GCN-normalized propagation) distributed Bass kernel for 8 TRN2 cores.

Strategy (dst-sharded message passing):
  - Nodes sharded across 8 cores (6250/core, padded to 6272 = 49*128 rows).
  - Prologue per core: atom-embedding gather (dma_gather from the flattened
    [9*119, 128] table) summed over 9 feature columns -> h0acc; 3-layer MLP
    on the TensorEngine (f32); produces h (bf16, tile layout) + h0s = 0.1*h
    (bf16) per shard.
  - Per-iteration exchange: each shard is split into an A half (rows 0:3200)
    and B half (3200:6272); two AllGathers produce full bf16 h copies in two
    Shared DRAM buffers of 25600/24576 rows (< 32768, so dma_gather's int16
    indices reach everything). Buffers are double-buffered across iterations
    and AG_a is issued mid-iteration (as soon as the A-half blends finish),
    so the collectives hide behind gather/matmul work.
  - Per iteration: dma_gather h[src] for every in-edge of the core's dst
    shard (incl. self-loops, which are ordinary edges with coef
    0.9*dinv^2), edges grouped by 64-wide dst block into 128-slot tiles;
    TensorEngine segment-sum via one-hot(0.9*coef) S matrices (bf16,
    SBUF-resident) accumulating in PSUM; h0s injected into PSUM with an
    identity matmul; the scalar engine evicts PSUM -> h (bf16) / out (f32).
    The vector engine does no per-edge work at all.
  - Per-block/domain tile counts are equalized across cores so all 8 cores
    run one SPMD instruction stream (padding tiles gather idx 0 with S=0).

kernel(**inputs) takes FULL inputs, shards on host, runs the NEFF on cores
0-7, and returns the full [50000, 128] f32 output.
"""

import numpy as np
import ml_dtypes

import concourse.bacc as bacc
import concourse.bass as bass
import concourse.mybir as mybir
import concourse.tile as tile
from concourse.bass_utils import run_bass_kernel_spmd

# Problem constants (hardcoded; must match reference.py)
N_NODES = 50000
N_EDGES = 800000
D = 128
NUM_ITER = 10
NUM_LAYER = 3
ALPHA = 0.1
NUM_ATOM_FEATS = 9
ATOM_VOCAB = 119

NC = 8
SHARD = N_NODES // NC            # 6250
SHARD_PAD = 6272                 # 49 * 128
NCOL = SHARD_PAD // 128          # 49
W = 64                           # dst block width
NBLK = SHARD_PAD // W            # 98
CHUNK = 1024                     # max idxs per dma_gather
HALF_A_ROWS = 3200               # shard rows in exchange buffer A (25 cols)
HALF_B_ROWS = 3072               # shard rows in exchange buffer B (24 cols)
N_A = NC * HALF_A_ROWS           # 25600 (< 32768 -> int16 safe)
N_B = NC * HALF_B_ROWS           # 24576 (< 32768 -> int16 safe)
COLS_A = HALF_A_ROWS // 128      # 25
NQ = 4                           # SWDGE queues

BF16 = mybir.dt.bfloat16
F32 = mybir.dt.float32
I16 = mybir.dt.int16
AF = mybir.ActivationFunctionType


def _wrap_idxs(idx):
    """slot i -> partition i%16 (replicated x8), col i//16."""
    n = idx.shape[0]
    assert n % 16 == 0
    w = idx.reshape(n // 16, 16).T.astype(np.int16)
    return np.ascontiguousarray(np.tile(w, (8, 1)))


def _pad128(a, fill=0):
    n = a.shape[0]
    m = (-n) % 128
    if m == 0:
        return a
    return np.concatenate([a, np.full((m,) + a.shape[1:], fill, a.dtype)])


def _preprocess(edge_index):
    """Host-side graph preprocessing -> per-core structures."""
    src = np.asarray(edge_index[0], dtype=np.int64)
    dst = np.asarray(edge_index[1], dtype=np.int64)
    deg = np.bincount(dst, minlength=N_NODES).astype(np.float64) + 1.0
    dinv = 1.0 / np.sqrt(deg)
    coef = ((1.0 - ALPHA) * dinv[src] * dinv[dst]).astype(np.float32)
    dinv2 = ((1.0 - ALPHA) * dinv * dinv).astype(np.float32)  # self-loop term
    rank = src // SHARD
    r = src % SHARD
    isa = r < HALF_A_ROWS
    srow = np.where(isa, rank * HALF_A_ROWS + r,
                    rank * HALF_B_ROWS + (r - HALF_A_ROWS))

    cores = []
    for c in range(NC):
        m = (dst >= c * SHARD) & (dst < (c + 1) * SHARD)
        nodes = np.arange(SHARD)
        self_isa = nodes < HALF_A_ROWS
        self_row = np.where(self_isa, c * HALF_A_ROWS + nodes,
                            c * HALF_B_ROWS + (nodes - HALF_A_ROWS))
        ldst = np.concatenate([(dst[m] - c * SHARD).astype(np.int64), nodes])
        lsrow = np.concatenate([srow[m], self_row])
        lcoef = np.concatenate([coef[m], dinv2[c * SHARD + nodes]])
        blk = ldst // W
        off = ldst % W
        islo = np.concatenate([isa[m], self_isa])

        streams = {"lo": [], "hi": []}     # list of idx arrays
        s_tiles = []                       # list of [128, W] f32 tile mats
        tiles_by_block = [[] for _ in range(NBLK)]  # (dom, tile_pos_in_stream)
        stream_ntiles = {"lo": 0, "hi": 0}
        for b in range(NBLK):
            bm = blk == b
            for dom, dm in (("lo", islo), ("hi", ~islo)):
                sel = bm & dm
                n = int(sel.sum())
                if n == 0:
                    continue
                idx = _pad128(lsrow[sel].astype(np.int64))
                cf = _pad128(lcoef[sel])
                of = _pad128(off[sel].astype(np.int64))
                ntile = idx.shape[0] // 128
                for t in range(ntile):
                    s = np.zeros((128, W), np.float32)
                    s[np.arange(128), of[t * 128:(t + 1) * 128]] = \
                        cf[t * 128:(t + 1) * 128]
                    tiles_by_block[b].append((dom, stream_ntiles[dom] + t,
                                              len(s_tiles)))
                    s_tiles.append(s)
                streams[dom].append(idx)
                stream_ntiles[dom] += ntile

        lo_idx = (np.concatenate(streams["lo"]) if streams["lo"]
                  else np.zeros(0, np.int64))
        hi_idx = (np.concatenate(streams["hi"]) if streams["hi"]
                  else np.zeros(0, np.int64))
        s_all = (np.stack(s_tiles) if s_tiles
                 else np.zeros((0, 128, W), np.float32))
        # S as SBUF layout [128, ntiles*W]
        s_sb = np.ascontiguousarray(
            s_all.transpose(1, 0, 2).reshape(128, -1)).astype(ml_dtypes.bfloat16)
        cores.append(dict(
            lo_idx=lo_idx, hi_idx=hi_idx, s_sb=s_sb,
            tiles_by_block=tiles_by_block,
            n_lo=lo_idx.shape[0], n_hi=hi_idx.shape[0],
            ntiles=len(s_tiles),
        ))
    return cores


def _chunks(total):
    """Split a stream of `total` slots (multiple of 128) into <=1024 chunks."""
    out = []
    o = 0
    while o < total:
        n = min(CHUNK, total - o)
        out.append((o, n))
        o += n
    return out


def _equalize(cores_meta):
    """Pad per-block/domain tile counts to the max across cores so all cores
    share one instruction stream. Padding tiles gather idx 0 with S=0."""
    # per core: per block, per dom tile count
    cnt = np.zeros((NC, NBLK, 2), np.int64)
    for c, m in enumerate(cores_meta):
        for b in range(NBLK):
            for dom, tpos, sidx in m["tiles_by_block"][b]:
                cnt[c, b, 0 if dom == "lo" else 1] += 1
    mx = cnt.max(axis=0)  # [NBLK, 2]

    new = []
    for c, m in enumerate(cores_meta):
        lo_parts, hi_parts, s_parts = [], [], []
        tiles_by_block = [[] for _ in range(NBLK)]
        lo_idx, hi_idx = m["lo_idx"], m["hi_idx"]
        # existing tiles grouped by block/dom in stream order
        ptr = {"lo": 0, "hi": 0}
        sidx_of = {}
        for b in range(NBLK):
            for dom, tpos, sidx in m["tiles_by_block"][b]:
                sidx_of[(b, dom, tpos)] = sidx
        s_all = m["s_sb"].reshape(128, -1, W)
        lo_nt, hi_nt = 0, 0
        s_n = 0
        for b in range(NBLK):
            for di, dom in enumerate(("lo", "hi")):
                have = [t for t in m["tiles_by_block"][b] if t[0] == dom]
                need = int(mx[b, di])
                for k in range(need):
                    if k < len(have):
                        _, tpos, sidx = have[k]
                        idx_arr = (lo_idx if dom == "lo" else hi_idx)[
                            tpos * 128:(tpos + 1) * 128]
                        s_mat = s_all[:, sidx, :]
                    else:
                        idx_arr = np.zeros(128, np.int64)
                        s_mat = np.zeros((128, W), ml_dtypes.bfloat16)
                    (lo_parts if dom == "lo" else hi_parts).append(idx_arr)
                    s_parts.append(np.asarray(s_mat))
                    nt = lo_nt if dom == "lo" else hi_nt
                    tiles_by_block[b].append((dom, nt, s_n))
                    s_n += 1
                    if dom == "lo":
                        lo_nt += 1
                    else:
                        hi_nt += 1
        lo_cat = (np.concatenate(lo_parts) if lo_parts
                  else np.zeros(0, np.int64))
        hi_cat = (np.concatenate(hi_parts) if hi_parts
                  else np.zeros(0, np.int64))
        s_cat = (np.stack(s_parts) if s_parts
                 else np.zeros((0, 128, W), ml_dtypes.bfloat16))
        s_sb = np.ascontiguousarray(
            np.asarray(s_cat).transpose(1, 0, 2).reshape(128, -1))
        new.append(dict(
            lo_idx=lo_cat, hi_idx=hi_cat, s_sb=s_sb,
            tiles_by_block=tiles_by_block,
            n_lo=lo_cat.shape[0], n_hi=hi_cat.shape[0], ntiles=s_n,
        ))
    return new


def _build_uniform(meta0):
    """Build the (identical-across-cores) program from equalized metadata."""
    n_lo, n_hi, ntiles = meta0["n_lo"], meta0["n_hi"], meta0["ntiles"]
    tiles_by_block = meta0["tiles_by_block"]
    n_emb = NUM_ATOM_FEATS * SHARD_PAD

    nc = bacc.Bacc("TRN2", target_bir_lowering=False, debug=False,
                   num_devices=NC, num_swdge_queues=NQ)

    emb = nc.dram_tensor("emb", [NUM_ATOM_FEATS * ATOM_VOCAB, D], F32,
                         kind="ExternalInput")
    emb_idx = nc.dram_tensor("emb_idx", [128, n_emb // 16], I16,
                             kind="ExternalInput")
    ws = nc.dram_tensor("ws", [NUM_LAYER * D, D], F32, kind="ExternalInput")
    bs = nc.dram_tensor("bs", [NUM_LAYER, D], F32, kind="ExternalInput")
    ident = nc.dram_tensor("ident", [128, 128], F32, kind="ExternalInput")
    idx_lo_d = nc.dram_tensor("idx_lo", [128, max(n_lo, 16) // 16], I16,
                              kind="ExternalInput")
    idx_hi_d = nc.dram_tensor("idx_hi", [128, max(n_hi, 16) // 16], I16,
                              kind="ExternalInput")
    s_d = nc.dram_tensor("s", [128, max(ntiles, 1) * W], BF16,
                         kind="ExternalInput")
    identb_d = nc.dram_tensor("identb", [128, 128], BF16,
                              kind="ExternalInput")
    out_d = nc.dram_tensor("out", [SHARD_PAD, D], F32, kind="ExternalOutput")

    ag_in_a = [nc.dram_tensor(f"ag_in_a{i}", [HALF_A_ROWS, D], BF16,
                              kind="Internal") for i in range(2)]
    ag_in_b = [nc.dram_tensor(f"ag_in_b{i}", [HALF_B_ROWS, D], BF16,
                              kind="Internal") for i in range(2)]
    ag_out_a = [nc.dram_tensor(f"ag_out_a{i}", [N_A, D], BF16,
                               kind="Internal", addr_space="Shared")
                for i in range(2)]
    ag_out_b = [nc.dram_tensor(f"ag_out_b{i}", [N_B, D], BF16,
                               kind="Internal", addr_space="Shared")
                for i in range(2)]

    def _emit_ag(which, buf):
        src = (ag_in_a if which == "a" else ag_in_b)[buf]
        dst = (ag_out_a if which == "a" else ag_out_b)[buf]
        nc.gpsimd.collective_compute(
            "AllGather", mybir.AluOpType.bypass,
            replica_groups=[list(range(NC))],
            ins=[src[:].opt()], outs=[dst[:].opt()])

    lo_chunks = _chunks(n_lo)
    hi_chunks = _chunks(n_hi)

    with tile.TileContext(nc) as tc:
      with tc.tile_pool(name="persist", bufs=1) as persist:
        # ---------------- prologue: embedding + MLP ----------------
        with (
            tc.tile_pool(name="pro", bufs=1) as pro,
            tc.tile_pool(name="embg", bufs=12) as embg,
            tc.tile_pool(name="prps", bufs=2, space="PSUM") as prps,
        ):
            h0acc = pro.tile([128, NCOL, D], F32, tag="h0acc")
            eidx = pro.tile([128, n_emb // 16], I16, tag="eidx")
            nc.sync.dma_start(eidx[:], emb_idx[:])
            idn = pro.tile([128, 128], F32, tag="idn")
            nc.sync.dma_start(idn[:], ident[:])
            # Sum the 9 per-feature embedding gathers on the TensorEngine
            # (identity-stationary matmuls accumulating in PSUM), chunk-major
            # so at most a couple of PSUM banks are live.
            qn = 0
            with tc.tile_pool(name="embp", bufs=4, space="PSUM") as embp:
                for (o, n) in _chunks(SHARD_PAD):
                    gs = []
                    for f in range(NUM_ATOM_FEATS):
                        g = embg.tile([128, 8, D], F32, tag="eg")
                        so = f * SHARD_PAD + o
                        nc.gpsimd.dma_gather(
                            g[:, 0:n // 128, :], emb[:, :],
                            eidx[:, so // 16:(so + n) // 16], n, n, D,
                            queue_num=qn % NQ)
                        qn += 1
                        gs.append(g)
                    for half in range(0, n // 128, 4):
                        w = min(4, n // 128 - half)
                        ep = embp.tile([128, 512], F32, tag="ep")
                        for f in range(NUM_ATOM_FEATS):
                            nc.tensor.matmul(
                                ep[:, 0:w * 128],
                                idn[:, :],
                                gs[f][:, half:half + w, :],
                                start=(f == 0),
                                stop=(f == NUM_ATOM_FEATS - 1))
                        nc.scalar.activation(
                            h0acc[:, o // 128 + half:o // 128 + half + w, :],
                            ep[:, 0:w * 128].rearrange(
                                "p (a b) -> p a b", b=128), AF.Copy)
            w_sb = pro.tile([128, NUM_LAYER * D], F32, tag="w")
            b_sb = pro.tile([128, NUM_LAYER], F32, tag="b")
            for l in range(NUM_LAYER):
                nc.sync.dma_start(w_sb[:, l * D:(l + 1) * D],
                                  ws[l * D:(l + 1) * D, :])
                nc.sync.dma_start(b_sb[:, l:l + 1],
                                  bs[l:l + 1, :].rearrange("a k -> k a"))

            hT = pro.tile([128, SHARD_PAD], F32, tag="hT")
            hT2 = pro.tile([128, SHARD_PAD], F32, tag="hT2")
            # transpose h0acc tiles into hT
            for cidx in range(NCOL):
                pt = prps.tile([128, 128], F32, tag="pt")
                nc.tensor.transpose(pt[:], h0acc[:, cidx, :], idn[:])
                nc.vector.tensor_copy(hT[:, cidx * 128:(cidx + 1) * 128],
                                      pt[:])
            # 3 MLP layers (f32)
            cur, nxt = hT, hT2
            mlp_chunks = [(o, min(512, SHARD_PAD - o))
                          for o in range(0, SHARD_PAD, 512)]
            for l in range(NUM_LAYER):
                for (o, n) in mlp_chunks:
                    ps = prps.tile([128, 512], F32, tag="mlp")
                    nc.tensor.matmul(ps[:, 0:n],
                                     w_sb[:, l * D:(l + 1) * D],
                                     cur[:, o:o + n], start=True, stop=True)
                    nc.scalar.activation(
                        nxt[:, o:o + n], ps[:, 0:n],
                        AF.Relu if l != NUM_LAYER - 1 else AF.Identity,
                        bias=b_sb[:, l:l + 1])
                cur, nxt = nxt, cur

            # transpose back; produce h (bf16) and h0s = 0.1*h (f32)
            h_sb = persist.tile([128, NCOL, D], BF16, tag="h")
            h0s = persist.tile([128, NCOL, D], BF16, tag="h0s")
            for cidx in range(NCOL):
                pt = prps.tile([128, 128], F32, tag="pt")
                nc.tensor.transpose(pt[:], cur[:, cidx * 128:(cidx + 1) * 128],
                                    idn[:])
                nc.vector.tensor_copy(h_sb[:, cidx, :], pt[:])
                nc.scalar.activation(h0s[:, cidx, :], pt[:], AF.Copy,
                                     scale=ALPHA)
            nc.sync.dma_start(
                ag_in_a[0][:].rearrange("(c p) f -> p c f", p=128),
                h_sb[:, 0:COLS_A, :])
            nc.sync.dma_start(
                ag_in_b[0][:].rearrange("(c p) f -> p c f", p=128),
                h_sb[:, COLS_A:NCOL, :])

        # ---------------- main loop ----------------
        with (
            tc.tile_pool(name="sconst", bufs=1) as sconst,
            tc.tile_pool(name="glo", bufs=8) as glo_pool,
            tc.tile_pool(name="ghi", bufs=8) as ghi_pool,
            tc.tile_pool(name="ps", bufs=8, space="PSUM") as ps_pool,
            tc.tile_pool(name="stage", bufs=4) as stage_pool,
        ):
            s_sb = sconst.tile([128, max(ntiles, 1) * W], BF16, tag="s")
            nc.sync.dma_start(s_sb[:], s_d[:])
            ilo = sconst.tile([128, max(n_lo, 16) // 16], I16, tag="ilo")
            nc.sync.dma_start(ilo[:], idx_lo_d[:])
            ihi = sconst.tile([128, max(n_hi, 16) // 16], I16, tag="ihi")
            nc.sync.dma_start(ihi[:], idx_hi_d[:])
            identb = sconst.tile([128, 128], BF16, tag="identb")
            nc.sync.dma_start(identb[:], identb_d[:])

            _emit_ag("a", 0)
            _emit_ag("b", 0)

            for it in range(NUM_ITER):
                buf = it % 2
                lo_view = ag_out_a[buf][:, :]
                hi_view = ag_out_b[buf][:, :]

                # issue gathers, interleaved a/b
                lo_tiles_bufs = {}
                hi_tiles_bufs = {}
                qn = 0
                li, hi_i = 0, 0
                order = []
                while li < len(lo_chunks) or hi_i < len(hi_chunks):
                    if li < len(lo_chunks):
                        order.append(("lo", li)); li += 1
                    if hi_i < len(hi_chunks):
                        order.append(("hi", hi_i)); hi_i += 1
                for dom, ci in order:
                    (o, n) = (lo_chunks if dom == "lo" else hi_chunks)[ci]
                    pool = glo_pool if dom == "lo" else ghi_pool
                    view = lo_view if dom == "lo" else hi_view
                    isb = ilo if dom == "lo" else ihi
                    g = pool.tile([128, 8, D], BF16, tag="g" + dom)
                    nc.gpsimd.dma_gather(
                        g[:, 0:n // 128, :], view,
                        isb[:, o // 16:(o + n) // 16], n, n, D,
                        queue_num=qn % NQ)
                    qn += 1
                    (lo_tiles_bufs if dom == "lo" else hi_tiles_bufs)[ci] = g

                # segment-sum matmuls + evict, block-pair by block-pair
                last = it == NUM_ITER - 1
                for p in range(NBLK // 2):
                    col = p
                    psum = ps_pool.tile([128, D], F32, tag="ps")
                    for half in range(2):
                        tl = tiles_by_block[2 * p + half]
                        ph = half * 64
                        # inject h0s (= 0.1*h0) via identity matmul
                        nc.tensor.matmul(
                            psum[ph:ph + 64, :],
                            identb[:, ph:ph + 64],
                            h0s[:, col, :],
                            start=True, stop=(len(tl) == 0))
                        for j, (dom, tpos, sidx) in enumerate(tl):
                            bufs = (lo_tiles_bufs if dom == "lo"
                                    else hi_tiles_bufs)
                            g = bufs[tpos // 8]
                            nc.tensor.matmul(
                                psum[ph:ph + 64, :],
                                s_sb[:, sidx * W:(sidx + 1) * W],
                                g[:, tpos % 8, :],
                                start=False, stop=(j == len(tl) - 1))
                    if last:
                        st = stage_pool.tile([128, D], F32, tag="st")
                        nc.scalar.activation(st[:], psum[:, :], AF.Copy)
                        nc.sync.dma_start(
                            out_d[p * 128:(p + 1) * 128, :], st[:])
                    else:
                        nc.scalar.activation(h_sb[:, col, :], psum[:, :],
                                             AF.Copy)
                    if not last:
                        if p == COLS_A - 1:
                            nc.sync.dma_start(
                                ag_in_a[1 - buf][:].rearrange(
                                    "(c p) f -> p c f", p=128),
                                h_sb[:, 0:COLS_A, :])
                            _emit_ag("a", 1 - buf)
                        elif p == NBLK // 2 - 1:
                            nc.sync.dma_start(
                                ag_in_b[1 - buf][:].rearrange(
                                    "(c p) f -> p c f", p=128),
                                h_sb[:, COLS_A:NCOL, :])
                            _emit_ag("b", 1 - buf)

    nc.compile()
    return nc


_CACHE = {}


def _get_compiled(edge_index):
    key = hash(np.asarray(edge_index).tobytes())
    if key not in _CACHE:
        cores = _preprocess(edge_index)
        cores = _equalize(cores)
        nc = _build_uniform(cores[0])
        _CACHE[key] = (nc, cores)
    return _CACHE[key]


def _make_in_maps(x, atom_emb, Ws, bs, cores_meta):
    x = np.asarray(x)
    emb_t = np.ascontiguousarray(
        np.asarray(atom_emb, dtype=np.float32).reshape(
            NUM_ATOM_FEATS * ATOM_VOCAB, D))
    ws_t = np.ascontiguousarray(
        np.asarray(Ws, dtype=np.float32).reshape(NUM_LAYER * D, D))
    bs_t = np.ascontiguousarray(np.asarray(bs, dtype=np.float32))
    ident = np.eye(128, dtype=np.float32)

    in_maps = []
    for c, m in enumerate(cores_meta):
        # embedding idx: per feature stream of SHARD_PAD slots
        ei = np.zeros(NUM_ATOM_FEATS * SHARD_PAD, np.int64)
        xs = x[c * SHARD:(c + 1) * SHARD]  # [SHARD, 9]
        for f in range(NUM_ATOM_FEATS):
            ei[f * SHARD_PAD:f * SHARD_PAD + SHARD] = \
                f * ATOM_VOCAB + xs[:, f]
        lo = m["lo_idx"] if m["n_lo"] else np.zeros(16, np.int64)
        hi_ = m["hi_idx"] if m["n_hi"] else np.zeros(16, np.int64)
        in_maps.append({
            "emb": emb_t,
            "emb_idx": _wrap_idxs(ei),
            "ws": ws_t,
            "bs": bs_t,
            "ident": ident,
            "idx_lo": _wrap_idxs(lo),
            "idx_hi": _wrap_idxs(hi_),
            "s": np.ascontiguousarray(m["s_sb"]),
            "identb": np.eye(128, dtype=ml_dtypes.bfloat16),
        })
    return in_maps


def kernel(x, edge_index, atom_emb, Ws, bs):
    nc, cores_meta = _get_compiled(edge_index)
    in_maps = _make_in_maps(x, atom_emb, Ws, bs, cores_meta)
    res = run_bass_kernel_spmd(nc, in_maps, core_ids=list(range(NC)))
    out = np.concatenate(
        [res.results[c]["out"][:SHARD] for c in range(NC)], axis=0)
    return np.ascontiguousarray(out.astype(np.float32))


def run_profiled(x, edge_index, atom_emb, Ws, bs):
    """Like kernel() but with NTFF profiling; returns (out, exec_time_ns)."""
    import ntff_hook
    ntff_hook.install()
    nc, cores_meta = _get_compiled(edge_index)
    in_maps = _make_in_maps(x, atom_emb, Ws, bs, cores_meta)
    res = run_bass_kernel_spmd(nc, in_maps, core_ids=list(range(NC)),
                               trace=True)
    out = np.concatenate(
        [res.results[c]["out"][:SHARD] for c in range(NC)], axis=0)
    return np.ascontiguousarray(out.astype(np.float32)), res.exec_time_ns
